# revision 6
# baseline (speedup 1.0000x reference)
"""BitMGQA fused kernel for 8 trn2 NeuronCores.

Sharding: core c handles batch b = c//2 and query-token half h = c%2.
Each core computes the full BitMGQA block for its 1024 query rows:
  - bit_linear projections (q/k/v) with exact integer-quantized matmuls
  - grouped-query attention (4 kv heads, q-head pairs pre-summed into weights)
  - LayerNorm + final bit_linear
k/v projections are computed for the full 2048-token batch on both cores of a
pair (duplicated) so no collectives are needed.

Quantization exactness trick: activation quant produces integers in [-127,127]
(exactly representable in fp16) and weight quant produces {-1,0,+1} signs, so
the matmuls accumulate exactly in fp32 PSUM at full fp16 PE rate; the
(weight-scale x per-token-scale) factors are applied on PSUM copyback.
round-half-even is implemented with the +1536 fp16 magic-constant trick.
"""

import os
import sys

import numpy as np

for _p in ("/opt/trn_rl_repo", "/root/.axon_site/_ro/trn_rl_repo"):
    if os.path.isdir(_p) and _p not in sys.path:
        sys.path.insert(0, _p)

import concourse.bacc as bacc
import concourse.bass as bass
import concourse.bass_isa as bass_isa
import concourse.mybir as mybir
import concourse.tile as tile
from concourse.bass_utils import run_bass_kernel_spmd

FP32 = mybir.dt.float32
FP16 = mybir.dt.float16
AX = mybir.AxisListType
ALU = mybir.AluOpType
ACT = mybir.ActivationFunctionType

# problem dims (per core)
NQ = 1024          # query tokens per core
NK = 2048          # key/value tokens per core
DIN = 1024         # embed dim
DKV = 512          # kv embed dim
H = 4              # kv heads
DH = 128           # head dim
NQT = NQ // 128    # 8 query token tiles
NKT = NK // 128    # 16 kv token tiles
RMS_EPS = 1e-6
LN_EPS = 1e-5
MAGIC = 1536.0     # fp16 round-to-int magic constant
BATCH = 4          # stats batching granularity (token tiles)


def _prep_weight(nc, pools, wT_dram, KO, DOUT_W, dest, eff_sum):
    """Stream wT (layout [KO*128, DOUT_W]) through stats, then sign-quantize
    into `dest` fp16. Returns wscale = mean|w| as [128,1] fp32 broadcast.
    eff_sum: dest gets sign(col block 2i) + sign(col block 2i+1) (q weights,
    kv-group pre-sum)."""
    st, wstage, wsgt = pools["stats"], pools["wstage"], pools["wsgt"]
    CW = 128
    NCH = DOUT_W // CW
    w3 = wT_dram.rearrange("(ko p) o -> p ko o", p=128)

    psums = st.tile([128, NCH], FP32, tag="wst", bufs=8, name="psums")
    asums = st.tile([128, NCH], FP32, tag="wst", bufs=8, name="asums")
    for ci in range(NCH):
        ch = wstage.tile([128, KO, CW], FP32, tag="wstage", bufs=3, name="wch")
        nc.sync.dma_start(ch[:], w3[:, :, ci * CW:(ci + 1) * CW])
        nc.vector.tensor_reduce(
            out=psums[:, ci:ci + 1], in_=ch[:], axis=AX.XY, op=ALU.add)
        nc.vector.tensor_reduce(
            out=asums[:, ci:ci + 1], in_=ch[:], axis=AX.XY, op=ALU.add,
            apply_absolute_value=True)
    comb = st.tile([128, 2], FP32, tag="wst", bufs=8, name="comb")
    nc.vector.tensor_reduce(out=comb[:, 0:1], in_=psums[:], axis=AX.X, op=ALU.add)
    nc.vector.tensor_reduce(out=comb[:, 1:2], in_=asums[:], axis=AX.X, op=ALU.add)
    allr = st.tile([128, 2], FP32, tag="wst", bufs=8, name="allr")
    nc.gpsimd.partition_all_reduce(
        allr[:], comb[:], channels=128, reduce_op=bass_isa.ReduceOp.add)
    nw = float(KO * 128 * DOUT_W)
    eneg = st.tile([128, 1], FP32, tag=f"eneg{DOUT_W}_{eff_sum}", bufs=1, name="eneg")
    nc.vector.tensor_scalar_mul(eneg[:], allr[:, 0:1], -1.0 / nw)
    wscale = st.tile([128, 1], FP32, tag=f"wsc{DOUT_W}_{eff_sum}", bufs=1, name="wscale")
    nc.vector.tensor_scalar_mul(wscale[:], allr[:, 1:2], 1.0 / nw)

    if eff_sum:
        # pairs of 128-col sign chunks summed into one dest block
        for ci in range(0, NCH, 2):
            t0 = wsgt.tile([128, KO, CW], FP16, tag="wsgt", bufs=4, name="sg0")
            t1 = wsgt.tile([128, KO, CW], FP16, tag="wsgt", bufs=4, name="sg1")
            for k, tgt in ((ci, t0), (ci + 1, t1)):
                ch = wstage.tile([128, KO, CW], FP32, tag="wstage", bufs=3, name="wch2")
                nc.sync.dma_start(ch[:], w3[:, :, k * CW:(k + 1) * CW])
                nc.scalar.activation(out=tgt[:], in_=ch[:], func=ACT.Sign, bias=eneg[:])
            nc.vector.tensor_tensor(
                out=dest[:, :, (ci // 2) * CW:(ci // 2 + 1) * CW],
                in0=t0[:], in1=t1[:], op=ALU.add)
    else:
        for ci in range(NCH):
            ch = wstage.tile([128, KO, CW], FP32, tag="wstage", bufs=3, name="wch3")
            nc.sync.dma_start(ch[:], w3[:, :, ci * CW:(ci + 1) * CW])
            nc.scalar.activation(
                out=dest[:, :, ci * CW:(ci + 1) * CW], in_=ch[:],
                func=ACT.Sign, bias=eneg[:])
    return wscale


def _quant_batch(nc, pools, xts, D, cs_dst, wscale, extra):
    """Quantize a batch of fp32 [128, D] tiles -> integer fp16 tiles.
    Writes combined copyback scale (mean|w| * 1/s_token * extra) columns into
    cs_dst [128, bn]. Returns list of int fp16 tiles."""
    st, xint = pools["stats"], pools["xint"]
    bn = len(xts)
    msq = st.tile([128, bn], FP32, tag="qst", bufs=20, name="msq")
    mabs = st.tile([128, bn], FP32, tag="qst", bufs=20, name="mabs")
    xqs = []
    for j, xt in enumerate(xts):
        xq = xint.tile([128, D], FP16, tag="xint", bufs=6, name="xq")
        nc.scalar.activation(out=xq[:], in_=xt[:], func=ACT.Square,
                             accum_out=msq[:, j:j + 1])
        nc.vector.tensor_reduce(out=mabs[:, j:j + 1], in_=xt[:], axis=AX.X,
                                op=ALU.max, apply_absolute_value=True)
        xqs.append(xq)
    msqn = st.tile([128, bn], FP32, tag="qst", bufs=20, name="msqn")
    nc.vector.tensor_scalar(msqn[:], msq[:], 1.0 / D, RMS_EPS, ALU.mult, ALU.add)
    sd = st.tile([128, bn], FP32, tag="qst", bufs=20, name="sdq")
    nc.scalar.activation(out=sd[:], in_=msqn[:], func=ACT.Sqrt)
    r = st.tile([128, bn], FP32, tag="qst", bufs=20, name="rq")
    nc.vector.reciprocal(r[:], sd[:])          # rsqrt(mean sq + eps)
    mn = st.tile([128, bn], FP32, tag="qst", bufs=20, name="mnq")
    nc.vector.tensor_tensor(out=mn[:], in0=mabs[:], in1=r[:], op=ALU.mult)
    sinv = st.tile([128, bn], FP32, tag="qst", bufs=20, name="sinv")
    nc.vector.tensor_scalar(sinv[:], mn[:], 1e-5, 1.0 / 127.0, ALU.max, ALU.mult)
    rec = st.tile([128, bn], FP32, tag="qst", bufs=20, name="recq")
    nc.vector.reciprocal(rec[:], sinv[:])
    alpha = st.tile([128, bn], FP32, tag="qst", bufs=20, name="alpha")
    nc.vector.tensor_tensor(out=alpha[:], in0=rec[:], in1=r[:], op=ALU.mult)
    if extra is not None:
        nc.vector.tensor_scalar(cs_dst[:], sinv[:], wscale[:, 0:1], extra,
                                ALU.mult, ALU.mult)
    else:
        nc.vector.tensor_scalar(cs_dst[:], sinv[:], wscale[:, 0:1], None,
                                ALU.mult)
    for j, (xt, xq) in enumerate(zip(xts, xqs)):
        # fp32->fp16 cast of (x*alpha + 1536) rounds to nearest int (RNE)
        nc.vector.tensor_scalar(
            xq[:], xt[:], alpha[:, j:j + 1], MAGIC, ALU.mult, ALU.add)
        nc.vector.tensor_scalar(xq[:], xq[:], MAGIC, None, ALU.subtract)
    return xqs


def _proj_tile(nc, pools, xq, KO, wT, DOUT_W, writer, t):
    """Token-major projection of one 128-token integer tile."""
    xT = pools["xT"].tile([128, KO, 128], FP16, tag="xT", bufs=6, name="xT")
    nc.sync.dma_start_transpose(out=xT[:], in_=xq[:])
    for oc in range((DOUT_W + 511) // 512):
        ow = min(512, DOUT_W - oc * 512)
        ps = pools["ppsum"].tile([128, 512], FP32, tag="ppsum", bufs=2, name="ps")
        for ko in range(KO):
            nc.tensor.matmul(
                ps[:, :ow], lhsT=xT[:, ko, :],
                rhs=wT[:, ko, oc * 512:oc * 512 + ow],
                start=(ko == 0), stop=(ko == KO - 1))
        writer(ps, t, oc, ow)


def build_nc():
    nc = bacc.Bacc("TRN2", target_bir_lowering=False, debug=False, num_devices=8)
    xq_d = nc.declare_dram_parameter("xq", [NQ, DIN], FP32, isOutput=False)
    xk_d = nc.declare_dram_parameter("xk", [NK, DIN], FP32, isOutput=False)
    xv_d = nc.declare_dram_parameter("xv", [NK, DIN], FP32, isOutput=False)
    wqT_d = nc.declare_dram_parameter("wqT", [DIN, DIN], FP32, isOutput=False)
    wkT_d = nc.declare_dram_parameter("wkT", [DIN, DKV], FP32, isOutput=False)
    wvT_d = nc.declare_dram_parameter("wvT", [DIN, DKV], FP32, isOutput=False)
    woT_d = nc.declare_dram_parameter("woT", [DKV, DIN], FP32, isOutput=False)
    lng_d = nc.declare_dram_parameter("lng", [DKV], FP32, isOutput=False)
    lnb_d = nc.declare_dram_parameter("lnb", [DKV], FP32, isOutput=False)
    y_d = nc.declare_dram_parameter("y", [NQ, DIN], FP32, isOutput=True)

    with tile.TileContext(nc) as tc:
        import contextlib
        ctx = contextlib.ExitStack()
        with ctx:
            pools = {}
            for nm, dflt in (("stats", 2), ("wstage", 3), ("wsgt", 4),
                             ("wpers", 1), ("xin", 5), ("xint", 6), ("xT", 6),
                             ("tokp", 6), ("attn", 1), ("P", 2), ("PT", 2),
                             ("xhat", 5), ("yout", 2)):
                pools[nm] = ctx.enter_context(tc.tile_pool(name=nm, bufs=dflt))
            for nm in ("ppsum", "spsum", "avpsum"):
                pools[nm] = ctx.enter_context(
                    tc.tile_pool(name=nm, bufs=2, space="PSUM"))

            st = pools["stats"]
            wpers = pools["wpers"]
            xin = pools["xin"]

            # ---- weight prep (sign quant + scales) ----
            wq_eff = wpers.tile([128, 8, DKV], FP16, tag="wq_eff", bufs=1)
            wk_s = wpers.tile([128, 8, DKV], FP16, tag="wk_s", bufs=1)
            wv_s = wpers.tile([128, 8, DKV], FP16, tag="wv_s", bufs=1)
            wo_s = wpers.tile([128, 4, DIN], FP16, tag="wo_s", bufs=1)
            aq = _prep_weight(nc, pools, wqT_d, 8, DIN, wq_eff, eff_sum=True)
            ak = _prep_weight(nc, pools, wkT_d, 8, DKV, wk_s, eff_sum=False)
            av = _prep_weight(nc, pools, wvT_d, 8, DKV, wv_s, eff_sum=False)
            ao = _prep_weight(nc, pools, woT_d, 4, DIN, wo_s, eff_sum=False)

            # gamma/beta broadcast rows
            gam = st.tile([128, DKV], FP32, tag="gam", bufs=1)
            bet = st.tile([128, DKV], FP32, tag="bet", bufs=1)
            nc.sync.dma_start(gam[:], lng_d[None, :].to_broadcast((128, DKV)))
            nc.sync.dma_start(bet[:], lnb_d[None, :].to_broadcast((128, DKV)))

            # persistent attention operands
            attn = pools["attn"]
            v_sb = attn.tile([128, NKT, DKV], FP16, tag="v_sb", bufs=1)
            qT = attn.tile([128, H, NQ], FP16, tag="qT", bufs=1)
            kT = attn.tile([128, H, NK], FP16, tag="kT", bufs=1)
            ao_sb = attn.tile([128, NQT, DKV], FP16, tag="ao_sb", bufs=1)

            cs_q = st.tile([128, NQT], FP32, tag="cs_q", bufs=1)
            cs_k = st.tile([128, NKT], FP32, tag="cs_k", bufs=1)
            cs_v = st.tile([128, NKT], FP32, tag="cs_v", bufs=1)

            tokp = pools["tokp"]

            def q_writer(ps, t, oc, ow):
                qtk = tokp.tile([128, DKV], FP16, tag="tokp", bufs=6, name="qtk")
                nc.vector.tensor_scalar(qtk[:], ps[:, :ow], cs_q[:, t:t + 1],
                                        None, ALU.mult)
                nc.sync.dma_start_transpose(
                    out=qT[:, :, t * 128:(t + 1) * 128], in_=qtk[:])

            def k_writer(ps, t, oc, ow):
                ktk = tokp.tile([128, DKV], FP16, tag="tokp", bufs=6, name="ktk")
                nc.vector.tensor_scalar(ktk[:], ps[:, :ow], cs_k[:, t:t + 1],
                                        None, ALU.mult)
                nc.sync.dma_start_transpose(
                    out=kT[:, :, t * 128:(t + 1) * 128], in_=ktk[:])

            def v_writer(ps, t, oc, ow):
                nc.vector.tensor_scalar(v_sb[:, t, :], ps[:, :ow],
                                        cs_v[:, t:t + 1], None, ALU.mult)

            # ---- q/k/v: load -> quantize -> project (pipelined per batch) ----
            for x_d, n_tiles, wT, KO, DOUT_W, cs, wsc, extra, writer in (
                    (xq_d, NQT, wq_eff, 8, DKV, cs_q, aq, 1.0 / 128.0, q_writer),
                    (xk_d, NKT, wk_s, 8, DKV, cs_k, ak, None, k_writer),
                    (xv_d, NKT, wv_s, 8, DKV, cs_v, av, None, v_writer)):
                for t0 in range(0, n_tiles, BATCH):
                    bn = min(BATCH, n_tiles - t0)
                    xts = []
                    for j in range(bn):
                        xt = xin.tile([128, DIN], FP32, tag="xin", bufs=5, name="xt")
                        nc.sync.dma_start(
                            xt[:], x_d[(t0 + j) * 128:(t0 + j + 1) * 128, :])
                        xts.append(xt)
                    xqs = _quant_batch(nc, pools, xts, DIN,
                                       cs[:, t0:t0 + bn], wsc, extra)
                    for j in range(bn):
                        _proj_tile(nc, pools, xqs[j], KO, wT, DOUT_W,
                                   writer, t0 + j)

            # ---- attention ----
            Pp, PTp = pools["P"], pools["PT"]
            spsum, avpsum = pools["spsum"], pools["avpsum"]
            for h in range(H):
                for qt in range(NQT):
                    Pt = Pp.tile([128, NK], FP16, tag="P", bufs=2, name="Pt")
                    dh = st.tile([128, 4], FP32, tag="dh", bufs=6, name="dh")
                    for sc in range(NK // 512):
                        sp = spsum.tile([128, 512], FP32, tag="spsum", bufs=2,
                                        name="sp")
                        nc.tensor.matmul(
                            sp[:], lhsT=qT[:, h, qt * 128:(qt + 1) * 128],
                            rhs=kT[:, h, sc * 512:(sc + 1) * 512],
                            start=True, stop=True)
                        nc.scalar.activation(
                            out=Pt[:, sc * 512:(sc + 1) * 512], in_=sp[:],
                            func=ACT.Exp, accum_out=dh[:, sc:sc + 1])
                    den = st.tile([128, 1], FP32, tag="dh", bufs=6, name="den")
                    nc.vector.tensor_reduce(out=den[:], in_=dh[:], axis=AX.X,
                                            op=ALU.add)
                    dri = st.tile([128, 1], FP32, tag="dh", bufs=6, name="dri")
                    nc.vector.reciprocal(dri[:], den[:])
                    PTt = PTp.tile([128, NKT, 128], FP16, tag="PT", bufs=2,
                                   name="PTt")
                    nc.sync.dma_start_transpose(out=PTt[:], in_=Pt[:])
                    avp = avpsum.tile([128, 128], FP32, tag="avpsum", bufs=2,
                                      name="avp")
                    for sc in range(NKT):
                        nc.tensor.matmul(
                            avp[:], lhsT=PTt[:, sc, :],
                            rhs=v_sb[:, sc, h * DH:(h + 1) * DH],
                            start=(sc == 0), stop=(sc == NKT - 1))
                    nc.vector.tensor_scalar(
                        ao_sb[:, qt, h * DH:(h + 1) * DH], avp[:], dri[:],
                        None, ALU.mult)

            # ---- LayerNorm stats ----
            xint = pools["xint"]
            mu = st.tile([128, NQT], FP32, tag="ln", bufs=14, name="mu")
            msqU = st.tile([128, NQT], FP32, tag="ln", bufs=14, name="msqU")
            for qt in range(NQT):
                nc.vector.tensor_reduce(out=mu[:, qt:qt + 1], in_=ao_sb[:, qt, :],
                                        axis=AX.X, op=ALU.add)
                dump = xint.tile([128, DKV], FP16, tag="lnd", bufs=2, name="dump")
                nc.scalar.activation(out=dump[:], in_=ao_sb[:, qt, :],
                                     func=ACT.Square, accum_out=msqU[:, qt:qt + 1])
            nc.vector.tensor_scalar_mul(mu[:], mu[:], 1.0 / DKV)
            var = st.tile([128, NQT], FP32, tag="ln", bufs=14, name="var")
            nc.vector.tensor_scalar(var[:], msqU[:], 1.0 / DKV, LN_EPS,
                                    ALU.mult, ALU.add)
            musq = st.tile([128, NQT], FP32, tag="ln", bufs=14, name="musq")
            nc.vector.tensor_tensor(out=musq[:], in0=mu[:], in1=mu[:], op=ALU.mult)
            nc.vector.tensor_tensor(out=var[:], in0=var[:], in1=musq[:],
                                    op=ALU.subtract)
            sdl = st.tile([128, NQT], FP32, tag="ln", bufs=14, name="sdl")
            nc.scalar.activation(out=sdl[:], in_=var[:], func=ACT.Sqrt)
            rln = st.tile([128, NQT], FP32, tag="ln", bufs=14, name="rln")
            nc.vector.reciprocal(rln[:], sdl[:])

            # ---- final bit_linear: xhat -> quant -> project -> y ----
            cs_o = st.tile([128, NQT], FP32, tag="cs_o", bufs=1)
            yout, ppsum = pools["yout"], pools["ppsum"]
            xhat_p = pools["xhat"]

            def y_writer(ps, t, oc, ow):
                yt = y_tiles[t % BATCH]
                nc.vector.tensor_scalar(yt[:, oc * 512:oc * 512 + ow],
                                        ps[:, :ow], cs_o[:, t:t + 1],
                                        None, ALU.mult)

            for t0 in range(0, NQT, BATCH):
                bn = min(BATCH, NQT - t0)
                xhs = []
                for j in range(bn):
                    qt = t0 + j
                    xh = xhat_p.tile([128, DKV], FP32, tag="xhat", bufs=5,
                                     name="xh")
                    nc.vector.tensor_scalar(xh[:], ao_sb[:, qt, :],
                                            mu[:, qt:qt + 1], rln[:, qt:qt + 1],
                                            ALU.subtract, ALU.mult)
                    nc.vector.tensor_tensor(out=xh[:], in0=xh[:], in1=gam[:],
                                            op=ALU.mult)
                    nc.vector.tensor_tensor(out=xh[:], in0=xh[:], in1=bet[:],
                                            op=ALU.add)
                    xhs.append(xh)
                xqs = _quant_batch(nc, pools, xhs, DKV,
                                   cs_o[:, t0:t0 + bn], ao, None)
                y_tiles = []
                for j in range(bn):
                    yt = yout.tile([128, DIN], FP32, tag="yout", bufs=3,
                                   name="yt")
                    y_tiles.append(yt)
                for j in range(bn):
                    _proj_tile(nc, pools, xqs[j], 4, wo_s, DIN, y_writer, t0 + j)
                    t = t0 + j
                    nc.sync.dma_start(y_d[t * 128:(t + 1) * 128, :],
                                      y_tiles[j][:])

    nc.compile()
    return nc


_NC_CACHE = None


def _get_nc():
    global _NC_CACHE
    if _NC_CACHE is None:
        _NC_CACHE = build_nc()
    return _NC_CACHE


def make_in_maps(query, key, value, q_w, k_w, v_w, out_w, ln_gamma, ln_beta):
    wqT = np.ascontiguousarray(np.asarray(q_w, np.float32).T)
    wkT = np.ascontiguousarray(np.asarray(k_w, np.float32).T)
    wvT = np.ascontiguousarray(np.asarray(v_w, np.float32).T)
    woT = np.ascontiguousarray(np.asarray(out_w, np.float32).T)
    lng = np.ascontiguousarray(np.asarray(ln_gamma, np.float32))
    lnb = np.ascontiguousarray(np.asarray(ln_beta, np.float32))
    query = np.asarray(query, np.float32)
    key = np.asarray(key, np.float32)
    value = np.asarray(value, np.float32)
    in_maps = []
    for c in range(8):
        b, hf = divmod(c, 2)
        in_maps.append({
            "xq": np.ascontiguousarray(query[b, hf * NQ:(hf + 1) * NQ]),
            "xk": np.ascontiguousarray(key[b]),
            "xv": np.ascontiguousarray(value[b]),
            "wqT": wqT, "wkT": wkT, "wvT": wvT, "woT": woT,
            "lng": lng, "lnb": lnb,
        })
    return in_maps


def kernel(query, key, value, q_w, k_w, v_w, out_w, ln_gamma, ln_beta):
    nc = _get_nc()
    in_maps = make_in_maps(query, key, value, q_w, k_w, v_w, out_w,
                           ln_gamma, ln_beta)
    res = run_bass_kernel_spmd(nc, in_maps, core_ids=list(range(8)))
    out = np.empty((4, 2048, 1024), np.float32)
    for c in range(8):
        b, hf = divmod(c, 2)
        out[b, hf * NQ:(hf + 1) * NQ] = res.results[c]["y"]
    return out


if __name__ == "__main__":
    nc = build_nc()
    print("build ok, instructions:",
          sum(len(b.bb.instructions) if hasattr(b, 'bb') else len(b.instructions)
              for b in nc.m.functions[0].blocks))
